# revision 18
# baseline (speedup 1.0000x reference)
"""Trainium2 Bass kernel for LDPC sum-product BP decoding (nn_BP_Decoder).

Takes FULL unsharded inputs (llr_demapper [1024, 2040] plus Tanner-graph
index arrays), data-parallel over the batch axis across 8 NeuronCores
(128 batch rows per core), returns the FULL [1024, 2040] float32 output.

tanh-domain formulation (v3): messages are carried as t = tanh(cv/2)
instead of cv, which makes the whole BP iteration table-free on ACT:
  - check->var: t_out = M * exclude-self-product(t_in)  (pure DVE mults,
    slab-major contiguous [128,340] fp16 slices)
  - var->check: tanh addition  p (+) q = (p+q)/(1+pq), with the reciprocal
    on the ACT Reciprocal table (~1.4us, one table all iteration long -- no
    tanh<->ln table switches, which cost ~7.6us each on this HW).
  - tanh(llr/2) is precomputed on the host; 2*atanh via ln(1+Mx)-ln(1-Mx)
    appears only once, in the epilogue.
Host-side numpy simulation of the full fp16 pipeline (including the 1e-4
denominator clamps and +-1 output clips) gives rel err ~1e-4 vs the fp32
reference (budget 2e-2).

Identity tS = tanh((llr + cv0 + cv1 + cv2)/2): the per-block c2v input is
tS (-) tau_b, computed post-scatter in block space.  tS is assembled as
(q (+) t1v) (+) t2v with q = A (+) tau0 precomputed in the previous
iteration's slack, so only ONE tanh-add sits between the s2 and s3
scatters (the serial hole in the GPSIMD pipeline).

The global sign flip of the reference cancels by oddness; scatter index
vectors are identical to the ln-domain kernel (slab-major relabeling).
"""
import functools
import numpy as np

import concourse.bacc as bacc
import concourse.tile as tile
import concourse.mybir as mybir
from concourse.tile_rust import add_dep_helper
from contextlib import ExitStack

F32 = mybir.dt.float32
F16 = mybir.dt.float16
I16 = mybir.dt.int16
AF = mybir.ActivationFunctionType
OP = mybir.AluOpType

N = 2040      # variables (and per-block edges)
NGRP = 340    # check groups per block
DC = 6        # check degree
N_CORES = 8
M_CLIP = float(np.float32(1.0) - np.float32(1e-7))
EPS = 1e-4


class _Body:
    """tanh-domain BP iteration body (shared by the unrolled kernel and the
    For_i benchmark)."""

    def __init__(self, nc, tc, pool, A, A1, A2, sidx, llr32=None):
        self.nc, self.tc, self.pool = nc, tc, pool
        self.iter_idx = 0

        def t16(tag):
            return pool.tile([128, N], F16, tag=tag, name=tag)

        self.t16 = t16
        self.A = t16("A")
        self.A1 = t16("A1")
        self.A2 = t16("A2")
        self.idx_s = pool.tile([128, 4 * N], I16, tag="idx_s", name="idx_s")
        nc.sync.dma_start(self.A[:], A)
        nc.sync.dma_start(self.A1[:], A1)
        nc.sync.dma_start(self.A2[:], A2)
        nc.sync.dma_start(self.idx_s[:], sidx)
        if llr32 is not None:
            self.llr32s = pool.tile([128, N], F32, tag="llr32s", name="llr32s")
            nc.sync.dma_start(self.llr32s[:], llr32)
        self.ix_inv1 = self.idx_s[:, 0 * N:1 * N]
        self.ix_inv2 = self.idx_s[:, 1 * N:2 * N]
        self.ix_perm1 = self.idx_s[:, 2 * N:3 * N]
        self.ix_perm2 = self.idx_s[:, 3 * N:4 * N]

        # messages and transported tiles
        self.tau0 = t16("tau0")
        self.tau1 = t16("tau1")
        self.tau2 = t16("tau2")
        self.t1v = t16("t1v")
        self.t2v = t16("t2v")
        self.x1t = t16("x1t")
        self.x2t = t16("x2t")
        self.q = t16("q")      # A (+) tau0 of previous generation
        self.u = t16("u")      # q (+) t1v
        self.u1 = t16("u1")    # A (+) t1v
        self.tS = t16("tS")
        self.w0 = t16("w0")
        self.w1 = t16("w1")
        self.w2 = t16("w2")
        # product scratch
        self.ps = [t16(f"ps{b}") for b in range(3)]
        self.ex = [t16(f"ex{b}") for b in range(3)]
        # four scratch sets for tanh-add/sub (m, d, r, s): v-side adds use
        # A/B, the block-1/2 sub paths get their own so consecutive
        # iterations' DVE work doesn't serialize on scratch WAR hazards.
        self.scrA = [t16(f"scrA{i}") for i in range(4)]
        self.scrB = [t16(f"scrB{i}") for i in range(4)]
        self.scrC = [t16(f"scrC{i}") for i in range(4)]
        self.scrD = [t16(f"scrD{i}") for i in range(4)]
        self.prev_scat = []

    def act_recip(self, out_ap, in_ap):
        eng = self.nc.scalar
        ins = [eng.lower_ap(in_ap)]
        for arg in (0.0, 1.0, 0.0):  # bias, scale, alpha
            ins.append(mybir.ImmediateValue(dtype=mybir.dt.float32, value=arg))
        return eng.add_instruction(mybir.InstActivation(
            name=self.nc.get_next_instruction_name(),
            func=AF.Reciprocal, ins=ins, outs=[eng.lower_ap(out_ap)]))

    def tadd(self, out, p, q, scr):
        """out = clip((p+q)/(1+pq)); denominator clamped at +EPS."""
        v = self.nc.vector
        m, d, r, s = scr
        v.tensor_tensor(m[:], p[:], q[:], OP.mult)
        v.tensor_scalar(d[:], m[:], 1.0, EPS, OP.add, OP.max)
        self.act_recip(r[:], d[:])
        v.tensor_tensor(s[:], p[:], q[:], OP.add)
        v.tensor_tensor(m[:], s[:], r[:], OP.mult)
        v.tensor_scalar(out[:], m[:], 1.0, -1.0, OP.min, OP.max)

    def tsub(self, out, tS, tau, scr):
        """out = clip((tS-tau)/(1-tS*tau)) via (tau-tS)/(tS*tau-1)."""
        v = self.nc.vector
        m, d, r, s = scr
        v.tensor_tensor(m[:], tS[:], tau[:], OP.mult)
        v.tensor_scalar(d[:], m[:], -1.0, -EPS, OP.add, OP.min)
        self.act_recip(r[:], d[:])
        v.tensor_tensor(s[:], tau[:], tS[:], OP.subtract)
        v.tensor_tensor(m[:], s[:], r[:], OP.mult)
        v.tensor_scalar(out[:], m[:], 1.0, -1.0, OP.min, OP.max)

    def prods(self, b, w, xout):
        """xout = M * exclude-self product over slab-major 6-groups of w."""
        v = self.nc.vector
        tm = [w[:, NGRP * k:NGRP * (k + 1)] for k in range(DC)]
        pre = [self.ps[b][:, NGRP * j:NGRP * (j + 1)] for j in range(3)]
        suf = [self.ps[b][:, NGRP * (3 + j):NGRP * (4 + j)] for j in range(3)]
        ex = [self.ex[b][:, NGRP * k:NGRP * (k + 1)] for k in range(DC)]
        v.tensor_tensor(pre[0], tm[0], tm[1], OP.mult)
        v.tensor_tensor(pre[1], pre[0], tm[2], OP.mult)
        v.tensor_tensor(pre[2], pre[1], tm[3], OP.mult)
        v.tensor_tensor(ex[5], pre[2], tm[4], OP.mult)
        v.tensor_tensor(suf[0], tm[5], tm[4], OP.mult)
        v.tensor_tensor(suf[1], suf[0], tm[3], OP.mult)
        v.tensor_tensor(suf[2], suf[1], tm[2], OP.mult)
        v.tensor_tensor(ex[0], suf[2], tm[1], OP.mult)
        v.tensor_tensor(ex[1], tm[0], suf[2], OP.mult)
        v.tensor_tensor(ex[2], pre[0], suf[1], OP.mult)
        v.tensor_tensor(ex[3], pre[1], suf[0], OP.mult)
        v.tensor_tensor(ex[4], pre[2], tm[5], OP.mult)
        v.tensor_scalar_mul(xout[:], self.ex[b][:], M_CLIP)

    def scat(self, dst, src, ix):
        return self.nc.gpsimd.local_scatter(dst[:], src[:], ix, channels=128,
                                            num_elems=N, num_idxs=N)

    def prologue(self):
        self.prods(1, self.A1, self.tau1)
        self.prods(2, self.A2, self.tau2)
        self.prods(0, self.A, self.tau0)
        self.tadd(self.q, self.A, self.tau0, self.scrA)

    def _b2_path(self):
        """block-2 c2v from the previous generation's s4 result."""
        self.tsub(self.w2, self.x2t, self.tau2, self.scrD)
        self.prods(2, self.w2, self.tau2)

    def iteration(self):
        s1 = self.scat(self.t1v, self.tau1, self.ix_perm1)
        # under s1: block-2 c2v of the previous generation (prologue covers
        # generation 0, so skip on the first iteration)
        if self.iter_idx:
            self._b2_path()
        s2 = self.scat(self.t2v, self.tau2, self.ix_perm2)
        # under s2: two tanh-adds on t1v
        self.tadd(self.u, self.q, self.t1v, self.scrA)
        self.tadd(self.u1, self.A, self.t1v, self.scrB)
        # serial hole: one tanh-add produces tS for the out-scatters
        self.tadd(self.tS, self.u, self.t2v, self.scrA)
        s3 = self.scat(self.x1t, self.tS, self.ix_inv1)
        # during s3: block-0 path for the NEXT generation
        self.tadd(self.w0, self.u1, self.t2v, self.scrB)
        self.prods(0, self.w0, self.tau0)
        self.tadd(self.q, self.A, self.tau0, self.scrB)
        s4 = self.scat(self.x2t, self.tS, self.ix_inv2)
        # during s4: block-1 c2v
        self.tsub(self.w1, self.x1t, self.tau1, self.scrC)
        self.prods(1, self.w1, self.tau1)
        scats = (self.prev_scat[-1:] if self.iter_idx else []) + [s1, s2, s3, s4]
        for a, b in zip(scats[1:], scats):
            add_dep_helper(a.ins, b.ins, sync=False, reason="pool order")
        self.prev_scat = [s4]
        self.iter_idx += 1

    def epilogue(self, out):
        nc = self.nc
        s1 = self.scat(self.t1v, self.tau1, self.ix_perm1)
        if self.iter_idx:
            self._b2_path()
        s2 = self.scat(self.t2v, self.tau2, self.ix_perm2)
        for a, b in zip([s1, s2], self.prev_scat + [s1]):
            add_dep_helper(a.ins, b.ins, sync=False, reason="pool order")
        # cv_i = ln(1+M*t) - ln(1-M*t) per message, then sum with fp32 llr
        # (reuse dead iteration tiles as scratch)
        lA, lB = self.scrA[0], self.scrA[1]
        cv0, cv1, cv2 = self.w0, self.w1, self.w2
        for tau, cv in ((self.tau0, cv0), (self.t1v, cv1), (self.t2v, cv2)):
            nc.scalar.activation(lA[:], tau[:], AF.Ln, scale=M_CLIP, bias=1.0)
            nc.scalar.activation(lB[:], tau[:], AF.Ln, scale=-M_CLIP, bias=1.0)
            nc.vector.tensor_tensor(cv[:], lA[:], lB[:], OP.subtract)
        nc.vector.tensor_tensor(cv0[:], cv0[:], cv1[:], OP.add)
        nc.vector.tensor_tensor(cv0[:], cv0[:], cv2[:], OP.add)
        S32 = self.pool.tile([128, N], F32, tag="S32", name="S32")
        nc.vector.tensor_tensor(S32[:], cv0[:], self.llr32s[:], OP.add)
        nc.sync.dma_start(out, S32[:])


def make_body(nc, tc, pool, A, A1, A2, sidx, llr32=None):
    return _Body(nc, tc, pool, A, A1, A2, sidx, llr32)


@functools.lru_cache(maxsize=2)
def _build_bp(nb_iter):
    nc = bacc.Bacc("TRN2", target_bir_lowering=False, debug=False,
                   enable_asserts=False, num_devices=N_CORES)
    A = nc.dram_tensor("llr", [128, N], F16, kind="ExternalInput").ap()
    A1 = nc.dram_tensor("llrp1", [128, N], F16, kind="ExternalInput").ap()
    A2 = nc.dram_tensor("llrp2", [128, N], F16, kind="ExternalInput").ap()
    llr32 = nc.dram_tensor("llr32", [128, N], F32, kind="ExternalInput").ap()
    sidx = nc.dram_tensor("sidx", [128, 4 * N], I16, kind="ExternalInput").ap()
    out = nc.dram_tensor("out", [128, N], F32, kind="ExternalOutput").ap()

    with tile.TileContext(nc) as tc, ExitStack() as ctx:
        pool = ctx.enter_context(tc.tile_pool(name="p", bufs=1))
        body = _Body(nc, tc, pool, A, A1, A2, sidx, llr32)
        body.prologue()
        for _ in range(nb_iter):
            body.iteration()
        body.epilogue(out)
    nc.compile()
    return nc


# --- host-side layout / index preparation ---------------------------------

def _slab(pos):
    """group-major edge position -> slab-major device position."""
    g, k = pos // DC, pos % DC
    return k * NGRP + g


@functools.lru_cache(maxsize=1)
def _prep_graph(vn_msg_key):
    vg = np.frombuffer(vn_msg_key, dtype=np.int64).reshape(N, 3)
    inv1 = vg[:, 1] - N          # b1 edge position of var v (group-major)
    inv2 = vg[:, 2] - 2 * N
    perm1 = np.argsort(inv1)     # var at b1 edge position j
    perm2 = np.argsort(inv2)

    pos = np.arange(N)
    sl = _slab(pos)              # group-major -> slab-major
    border = np.empty(N, np.int64)   # slab-major device pos -> group-major
    border[sl] = pos

    vpos = sl                    # var v -> v-space device position (b0 slab)
    vorder = border              # device position -> var (b0: var == position)
    bpos1 = sl                   # b1 edge j -> b1 device position
    bpos2 = sl

    ix1 = vpos[perm1[border]]
    ix2 = vpos[perm2[border]]
    ix3 = bpos1[inv1[vorder]]
    ix4 = bpos2[inv2[vorder]]
    sidx = np.concatenate([ix3, ix4, ix1, ix2]).astype(np.int16)
    return (perm1, perm2, vpos, vorder, border,
            np.ascontiguousarray(np.tile(sidx[None, :], (128, 1))))


def _host_inputs(llr, vn_msg_ind):
    (perm1, perm2, vpos, vorder, border, sidx) = _prep_graph(
        np.asarray(vn_msg_ind, dtype=np.int64).tobytes())
    A = np.tanh(0.5 * llr)
    Av = np.ascontiguousarray(A[:, vorder]).astype(np.float16)
    A1 = np.ascontiguousarray(A[:, perm1[border]]).astype(np.float16)
    A2 = np.ascontiguousarray(A[:, perm2[border]]).astype(np.float16)
    lv32 = np.ascontiguousarray(llr[:, vorder]).astype(np.float32)
    return Av, A1, A2, lv32, sidx, vorder


class _Runner:
    """jit-compiled PJRT executor for a prebuilt Bass module on 8 cores."""

    def __init__(self, nc):
        import jax
        from jax.sharding import Mesh, PartitionSpec
        from jax.experimental.shard_map import shard_map
        from concourse.bass2jax import (_bass_exec_p, install_neuronx_cc_hook,
                                        partition_id_tensor)
        install_neuronx_cc_hook()
        self.jax = jax
        partition_name = (nc.partition_id_tensor.name
                          if nc.partition_id_tensor else None)
        in_names, out_names, out_avals, zero_outs = [], [], [], []
        for alloc in nc.m.functions[0].allocations:
            if not isinstance(alloc, mybir.MemoryLocationSet):
                continue
            name = alloc.memorylocations[0].name
            if alloc.kind == "ExternalInput":
                if name != partition_name:
                    in_names.append(name)
            elif alloc.kind == "ExternalOutput":
                out_names.append(name)
                shape = tuple(alloc.tensor_shape)
                dtype = mybir.dt.np(alloc.dtype)
                out_avals.append(jax.core.ShapedArray(shape, dtype))
                zero_outs.append(np.zeros(shape, dtype))
        self.in_names, self.out_names = in_names, out_names
        self.out_avals, self.zero_outs = out_avals, zero_outs
        n_params, n_outs = len(in_names), len(out_avals)
        all_in = tuple(in_names + out_names
                       + ([partition_name] if partition_name else []))
        donate = tuple(range(n_params, n_params + n_outs))

        def _body(*args):
            operands = list(args)
            if partition_name is not None:
                operands.append(partition_id_tensor())
            return tuple(_bass_exec_p.bind(
                *operands, out_avals=tuple(out_avals), in_names=all_in,
                out_names=tuple(out_names), lowering_input_output_aliases=(),
                sim_require_finite=True, sim_require_nnan=True, nc=nc))

        devices = jax.devices()[:N_CORES]
        mesh = Mesh(np.asarray(devices), ("core",))
        self.fn = jax.jit(
            shard_map(_body, mesh=mesh,
                      in_specs=(PartitionSpec("core"),) * (n_params + n_outs),
                      out_specs=(PartitionSpec("core"),) * n_outs,
                      check_rep=False),
            donate_argnums=donate, keep_unused=True)

    def run(self, in_maps):
        per_core = [[np.asarray(m[n]) for n in self.in_names] for m in in_maps]
        args = [np.concatenate([per_core[c][i] for c in range(N_CORES)], axis=0)
                for i in range(len(self.in_names))]
        args += [np.zeros((N_CORES * z.shape[0], *z.shape[1:]), z.dtype)
                 for z in self.zero_outs]
        outs = self.fn(*[self.jax.numpy.asarray(a) for a in args])
        self.jax.block_until_ready(outs)
        return [{n: np.asarray(outs[i]).reshape(N_CORES, *self.out_avals[i].shape)[c]
                 for i, n in enumerate(self.out_names)} for c in range(N_CORES)]


_runner_cache = {}


def _get_runner(nb_iter):
    if nb_iter not in _runner_cache:
        _runner_cache[nb_iter] = _Runner(_build_bp(nb_iter))
    return _runner_cache[nb_iter]


def kernel(llr_demapper, cn_msg_ind, vn_msg_ind, vn2cn_ind, cn_mask_ind,
           vn_mask_ind, edge_vn, nb_iter):
    llr = np.asarray(llr_demapper, dtype=np.float32)
    B = llr.shape[0]
    assert llr.shape == (B, N) and B % N_CORES == 0
    nb_iter = int(np.asarray(nb_iter))

    Av, A1, A2, lv32, sidx, vorder = _host_inputs(llr, vn_msg_ind)

    rows = B // N_CORES
    assert rows == 128, "kernel is specialized for 128 batch rows per core"
    in_maps = []
    for c in range(N_CORES):
        sl = slice(c * rows, (c + 1) * rows)
        in_maps.append({
            "llr": Av[sl],
            "llrp1": A1[sl],
            "llrp2": A2[sl],
            "llr32": lv32[sl],
            "sidx": sidx,
        })

    runner = _get_runner(nb_iter)
    res = runner.run(in_maps)
    dev_out = np.concatenate([r["out"] for r in res], axis=0)
    out = np.empty_like(dev_out)
    out[:, vorder] = dev_out          # undo the v-space slab layout
    return out


def make_bench_inputs():
    """Synthetic single-core inputs for bench_bp.py."""
    rng = np.random.default_rng(0)
    llr = rng.standard_normal((128, N)).astype(np.float32)
    p1 = rng.permutation(N)
    p2 = rng.permutation(N)
    vg = np.stack([np.arange(N), np.argsort(p1) + N,
                   np.argsort(p2) + 2 * N], axis=1)
    Av, A1, A2, lv32, sidx, vorder = _host_inputs(llr, vg.reshape(-1))
    return {"llr": Av, "llrp1": A1, "llrp2": A2, "llr32": lv32, "sidx": sidx}
